# revision 1
# baseline (speedup 1.0000x reference)
"""Trainium2 Bass kernel for NeuralGraphHidden (GNN message passing).

Full-input contract: kernel(**inputs) takes the complete unsharded arrays,
shards batch dim 0 across 8 NeuronCores (data parallel), runs one SPMD Bass
program, and reassembles the full output.

Math (per molecule, A=128 atoms, D=5 degree slots):
  deg[a]      = #(edges[a,:] != -1)
  neigh[a]    = atoms[a] + sum_d atoms[edges[a,d]]        (-1 -> no contribution)
  sbond[a]    = sum_d bonds[a,d]
  feat[a]     = [neigh[a], sbond[a], 1.0]                 (bias folded as feature)
  Z_d[a]      = feat[a] @ Waug[d]                          (Waug = [W;b])
  out[a]      = relu(Z_{deg[a]}[a])  if deg[a] < 5 else 0

On-chip formulation:
  ET'[a',a] = I + sum_d onehot(edges[a,d])[a']   built via DVE is_equal vs iota
  neighT    = atoms_chunk.T @ ET'                (TensorE, contracts atoms axis)
  deg+1     = ones.T-col-sums of ET'             (TensorE)
  bondsT    = 5 accumulating transposes          (TensorE, rhs=I)
  Z         = featT.T @ Waug                     (TensorE, 3 K-chunks x 5 degrees)
  select    = sum_d diag(deg==d) @ Z_d           (TensorE, PSUM-accumulated;
                                                  exact: relu and select commute)
  out       = relu(select)                       (ScalarE)
"""

import sys

sys.path.insert(0, "/opt/trn_rl_repo")

import numpy as np

B, A, D = 256, 128, 5
FA, FB, C = 256, 64, 256
F = FA + FB        # 320
FAUG = F + 1       # 321 (bias row)
NCORES = 8
BL = B // NCORES   # 32 molecules per core

_CACHE = {}


def _build_program():
    from contextlib import ExitStack

    import concourse.bass as bass
    import concourse.tile as tile
    from concourse import bacc, mybir

    f32 = mybir.dt.float32
    i32 = mybir.dt.int32
    AF = mybir.ActivationFunctionType
    OP = mybir.AluOpType
    # float32r operands: single-pass (TF32-like) PE matmul at 2x fp32 rate;
    # every matmul operand below is produced/declared as f32r to satisfy the
    # BIR verifier's rounding rule. One-hot/mask/count values are small
    # integers, exactly representable at reduced mantissa, so the gather and
    # degree-select logic stays exact.
    f32r = mybir.dt.float32r
    bf16 = mybir.dt.bfloat16

    nc = bacc.Bacc("TRN2", target_bir_lowering=False, debug=False,
                   num_devices=NCORES)

    atoms_d = nc.dram_tensor("atoms", [BL, A, FA], f32r, kind="ExternalInput")
    bonds_d = nc.dram_tensor("bonds", [BL, A, D * FB], f32,
                             kind="ExternalInput")
    # edge indices as bf16 (exact for -1..127), host-replicated down
    # partitions in (d, a)-major order for the 2x-mode add tree
    edges_d = nc.dram_tensor("edges", [BL, A, A * D], bf16,
                             kind="ExternalInput")
    waug_d = nc.dram_tensor("waug", [D, FAUG, C], f32r, kind="ExternalInput")
    ident_d = nc.dram_tensor("ident", [A, A], f32, kind="ExternalInput")
    identr_d = nc.dram_tensor("identr", [A, A], f32r, kind="ExternalInput")
    identb_d = nc.dram_tensor("identb", [A, A], bf16, kind="ExternalInput")
    iota_d = nc.dram_tensor("iota", [A, 1], f32, kind="ExternalInput")
    edeg_d = nc.dram_tensor("edeg", [BL, A, D], f32, kind="ExternalInput")
    onesr_d = nc.dram_tensor("onesr", [1, A], f32, kind="ExternalInput")
    out_d = nc.dram_tensor("out", [BL, A, C], f32, kind="ExternalOutput")

    atoms_ap = atoms_d.ap()
    bonds_ap = bonds_d.ap()
    edges_ap = edges_d.ap()
    out_ap = out_d.ap()

    with tile.TileContext(nc) as tc, ExitStack() as ctx:
        consts = ctx.enter_context(tc.tile_pool(name="consts", bufs=1))
        pin = ctx.enter_context(tc.tile_pool(name="pin", bufs=3))
        pbc = ctx.enter_context(tc.tile_pool(name="pbc", bufs=2))
        pet = ctx.enter_context(tc.tile_pool(name="pet", bufs=2))
        pfeat = ctx.enter_context(tc.tile_pool(name="pfeat", bufs=2))
        pmd = ctx.enter_context(tc.tile_pool(name="pmd", bufs=2))
        pz = ctx.enter_context(tc.tile_pool(name="pz", bufs=2))
        pout = ctx.enter_context(tc.tile_pool(name="pout", bufs=3))
        ps_f = ctx.enter_context(
            tc.tile_pool(name="ps_f", bufs=2, space="PSUM"))
        ps_c2 = ctx.enter_context(
            tc.tile_pool(name="ps_c2", bufs=1, space="PSUM"))
        ps_z = ctx.enter_context(
            tc.tile_pool(name="ps_z", bufs=1, space="PSUM"))
        ps_s = ctx.enter_context(
            tc.tile_pool(name="ps_s", bufs=1, space="PSUM"))

        # ---- one-time setup -------------------------------------------------
        # Constants + weights issue from the Scalar/GpSimd engine queues so
        # the Sync queue serves only per-group input loads (startup latency).
        G = 4
        ident = consts.tile([A, A], f32)
        nc.scalar.dma_start(out=ident[:], in_=ident_d.ap()[:])
        identr = consts.tile([A, A], f32r)
        nc.scalar.dma_start(out=identr[:], in_=identr_d.ap()[:])
        iota_col = consts.tile([A, 1], f32)
        nc.gpsimd.dma_start(out=iota_col[:], in_=iota_d.ap()[:])
        ones_row = consts.tile([1, A], f32)
        nc.scalar.dma_start(out=ones_row[:], in_=onesr_d.ap()[:])
        identb4 = consts.tile([A, G * A], bf16)
        for j in range(G):
            nc.gpsimd.dma_start(out=identb4[:, j * A:(j + 1) * A],
                                in_=identb_d.ap()[:])

        # Weights resident in SBUF: chunk k holds rows [k*128, ...) of Waug
        # for all 5 degrees side by side: w_k[:, d*256:(d+1)*256].
        # Issued from the Scalar engine queue so they don't delay the first
        # group's input loads on the Sync queue.
        w0 = consts.tile([128, D * C], f32r)
        w1 = consts.tile([128, D * C], f32r)
        w2 = consts.tile([FAUG - 256, D * C], f32r)  # 65 rows: 64 bond + bias
        for d in range(D):
            nc.scalar.dma_start(out=w0[:, d * C:(d + 1) * C],
                                in_=waug_d.ap()[d, 0:128, :])
            nc.scalar.dma_start(out=w1[:, d * C:(d + 1) * C],
                                in_=waug_d.ap()[d, 128:256, :])
            nc.scalar.dma_start(out=w2[:, d * C:(d + 1) * C],
                                in_=waug_d.ap()[d, 256:FAUG, :])

        # ---- molecules, in groups of 4 ----------------------------------
        # One DMA per group tensor (cuts SP issue cost 4x), batched DVE
        # compare/add-tree across the group, per-molecule matmul stages.
        for bg in range(BL // G):
            mols = range(bg * G, (bg + 1) * G)
            atoms4 = pin.tile([A, G * FA], f32r)
            nc.sync.dma_start(
                out=atoms4.rearrange("p (g f) -> p g f", g=G),
                in_=atoms_ap[bg * G:(bg + 1) * G].rearrange(
                    "g p f -> p g f"))
            bonds4 = pin.tile([A, G * D * FB], f32)
            nc.sync.dma_start(
                out=bonds4.rearrange("p (g f) -> p g f", g=G),
                in_=bonds_ap[bg * G:(bg + 1) * G].rearrange(
                    "g p f -> p g f"))
            bc_e4 = pbc.tile([A, G * A * D], bf16)
            nc.gpsimd.dma_start(
                out=bc_e4.rearrange("p (g f) -> p g f", g=G),
                in_=edges_ap[bg * G:(bg + 1) * G].rearrange(
                    "g p f -> p g f"))
            edeg4 = pfeat.tile([A, G * D], f32)
            nc.sync.dma_start(
                out=edeg4.rearrange("p (g f) -> p g f", g=G),
                in_=edeg_d.ap()[bg * G:(bg + 1) * G].rearrange(
                    "g p f -> p g f"))
            # deg+1 per molecule from the raw edge slots (tiny DVE chain)
            ne4 = pfeat.tile([A, G * D], f32)
            nc.vector.tensor_scalar(ne4[:], edeg4[:], -1.0, None,
                                    OP.not_equal)
            degp1_4 = pfeat.tile([A, G], f32)
            nc.vector.tensor_reduce(
                degp1_4[:], ne4.rearrange("p (g d) -> p g d", g=G),
                axis=mybir.AxisListType.X, op=OP.add)
            nc.vector.tensor_scalar(degp1_4[:], degp1_4[:], 1.0, None,
                                    OP.add)

            # one-hot compare + degree-slot sum (bf16 2x-mode add tree;
            # counts <= 6 are bf16-exact) + self term, batched per group
            cmp5 = pbc.tile([A, G * A * D], bf16)
            nc.vector.tensor_scalar(cmp5[:], bc_e4[:], iota_col[:], None,
                                    OP.is_equal)
            cg = cmp5.rearrange("p (g d a) -> p g d a", g=G, d=D)
            t01 = pet.tile([A, G * A], bf16)
            nc.vector.tensor_add(t01[:], cg[:, :, 0, :], cg[:, :, 1, :])
            t23 = pet.tile([A, G * A], bf16)
            nc.vector.tensor_add(t23[:], cg[:, :, 2, :], cg[:, :, 3, :])
            t4i = pet.tile([A, G * A], bf16)
            nc.vector.tensor_add(t4i[:], cg[:, :, 4, :], identb4[:])
            t0123 = pet.tile([A, G * A], bf16)
            nc.vector.tensor_add(t0123[:], t01[:], t23[:])
            etp4 = pet.tile([A, G * A], f32r)
            with nc.allow_low_precision(reason="exact small-int counts"):
                nc.vector.tensor_add(etp4[:], t0123[:], t4i[:])

            out4 = pout.tile([A, G * C], f32)
            for j, bm in enumerate(mols):
                etp = etp4[:, j * A:(j + 1) * A]
                atoms_sb = atoms4[:, j * FA:(j + 1) * FA]
                bonds_sb = bonds4[:, j * D * FB:(j + 1) * D * FB]

                degp1 = degp1_4[:, j:j + 1]

                # Neighbor+self sums (transposed) in one PSUM tile.
                pf = ps_f.tile([A, FA], f32)
                nc.tensor.matmul(pf[:, 0:128], atoms_sb[:, 0:128], etp)
                nc.tensor.matmul(pf[:, 128:256], atoms_sb[:, 128:256], etp)

                featT01 = pfeat.tile([A, FA], f32r)
                nc.scalar.copy(featT01[:], pf[:, 0:FA])

                # Bond sums on DVE, then one transpose matmul -> (fb, a).
                sumbond = pfeat.tile([A, FB], f32r)
                with nc.allow_low_precision(
                        reason="f32r rounding of bond sums"):
                    nc.vector.reduce_sum(
                        sumbond[:],
                        bonds_sb.rearrange("p (d f) -> p f d", d=D),
                        axis=mybir.AxisListType.X)
                pc2 = ps_c2.tile([FB, A], f32)
                nc.tensor.matmul(pc2[:], sumbond[:], identr[:])
                chunk2 = pfeat.tile([FAUG - 256, A], f32r)
                nc.scalar.copy(chunk2[0:FB, :], pc2[:])
                nc.vector.tensor_copy(chunk2[FB:FB + 1, :], ones_row[:])

                # maskdiag_d = diag(deg == d): (I*(deg+1)) == (d+1).
                md = pmd.tile([A, D * A], f32r)
                for d in range(D):
                    nc.vector.tensor_scalar(md[:, d * A:(d + 1) * A],
                                            ident[:], degp1[:], float(d + 1),
                                            OP.mult, OP.is_equal)

                # Dense: Z[:, d*256:(d+1)*256] = feat @ Waug[d].
                lhs = [featT01[:, 0:128], featT01[:, 128:256], chunk2[:]]
                rhs = [w0, w1, w2]
                groups = [(0, 512), (512, 1024), (1024, 1280)]
                zsb = pz.tile([A, D * C], f32r)
                for g0, g1 in groups:
                    pzg = ps_z.tile([A, 512], f32, tag="pzg", bufs=4)
                    nc.tensor.matmul(pzg[:, 0:g1 - g0], lhs[0],
                                     rhs[0][:, g0:g1], start=True, stop=False)
                    nc.tensor.matmul(pzg[:, 0:g1 - g0], lhs[1],
                                     rhs[1][:, g0:g1], start=False,
                                     stop=False)
                    nc.tensor.matmul(pzg[:, 0:g1 - g0], lhs[2],
                                     rhs[2][:, g0:g1], start=False, stop=True)
                    nc.scalar.copy(zsb[:, g0:g1], pzg[:, 0:g1 - g0])

                # Degree select, then one relu into the group output tile.
                pst = ps_s.tile([A, C], f32)
                for d in range(D):
                    nc.tensor.matmul(pst[:], md[:, d * A:(d + 1) * A],
                                     zsb[:, d * C:(d + 1) * C],
                                     start=(d == 0), stop=(d == D - 1))
                nc.scalar.activation(out4[:, j * C:(j + 1) * C], pst[:],
                                     AF.Relu)
            nc.gpsimd.dma_start(
                out=out_ap[bg * G:(bg + 1) * G].rearrange("g p f -> p g f"),
                in_=out4.rearrange("p (g f) -> p g f", g=G))

    nc.compile()
    return nc


def _get_nc():
    if "nc" not in _CACHE:
        _CACHE["nc"] = _build_program()
    return _CACHE["nc"]


def _make_in_maps(atoms, bonds, edges, W, b):
    atoms = np.ascontiguousarray(np.asarray(atoms, dtype=np.float32))
    bonds = np.ascontiguousarray(np.asarray(bonds, dtype=np.float32))
    edges = np.asarray(edges)
    W = np.asarray(W, dtype=np.float32)
    b = np.asarray(b, dtype=np.float32)

    # bf16 edge slots (exact for -1..127) replicated down the partition axis
    # (layout prep for the on-chip one-hot compare; DMA cannot zero-step
    # partitions).
    import ml_dtypes
    edges_f = np.ascontiguousarray(edges.transpose(0, 2, 1)).reshape(
        B, D * A).astype(ml_dtypes.bfloat16)
    edges_rep = np.ascontiguousarray(
        np.broadcast_to(edges_f[:, None, :], (B, A, D * A)))

    waug = np.ascontiguousarray(
        np.concatenate([W, b[:, None, :]], axis=1))           # (5, 321, 256)
    ident = np.eye(A, dtype=np.float32)
    iota = np.arange(A, dtype=np.float32).reshape(A, 1)
    onesr = np.ones((1, A), dtype=np.float32)

    edeg8 = edges.reshape(NCORES, BL, A, D).astype(np.float32)
    atoms8 = atoms.reshape(NCORES, BL, A, FA)
    bonds8 = bonds.reshape(NCORES, BL, A, D * FB)
    edges8 = edges_rep.reshape(NCORES, BL, A, A * D)

    return [
        {
            "atoms": atoms8[c],
            "bonds": bonds8[c],
            "edges": edges8[c],
            "waug": waug,
            "ident": ident,
            "identr": ident,
            "identb": ident.astype(ml_dtypes.bfloat16),
            "iota": iota,
            "edeg": edeg8[c],
            "onesr": onesr,
        }
        for c in range(NCORES)
    ]


def run_sharded(atoms, bonds, edges, W, b, trace=False):
    """Run on the 8 NeuronCores; returns (output, BassKernelResults)."""
    from concourse.bass_utils import run_bass_kernel_spmd

    nc = _get_nc()
    in_maps = _make_in_maps(atoms, bonds, edges, W, b)
    res = run_bass_kernel_spmd(nc, in_maps, list(range(NCORES)), trace=trace)
    out = np.concatenate([res.results[c]["out"] for c in range(NCORES)],
                         axis=0).reshape(B, A, C)
    return out, res


def kernel(atoms, bonds, edges, W, b):
    out, _ = run_sharded(atoms, bonds, edges, W, b)
    return out



# revision 24
# speedup vs baseline: 2.2339x; 2.2339x over previous
"""Trainium2 Bass kernel for NeuralGraphHidden (GNN message passing).

Full-input contract: kernel(**inputs) takes the complete unsharded arrays,
shards batch dim 0 across 8 NeuronCores (data parallel), runs one SPMD Bass
program, and reassembles the full output.

Key structural fact exploited: deg[a] = #(edges[a,:] != -1) is in 0..5, but
the reference's degree mask covers only 0..4 - atoms with deg==5 (about 96%
of atoms for this input distribution) produce an all-zero output row.  The
kernel compacts the few deg<5 atoms per molecule into static per-degree
slots on-chip and runs the dense layer only on those slots:

  per group of 8 molecules (4 groups/core, 32 molecules/core):
    deg       = row counts of edges != -1                  (Pool/DVE)
    rank_d    = per-degree prefix sums via tri-matmul       (PE)
    P         = slot one-hot (atom -> slot), 32-padded      (Pool)
    gatt      = per-slot edge ids via edeg^T @ P            (PE)
    E_j       = edge ids broadcast down partitions          (PE)
    ET        = sum_j onehot(E_j) + P  (neighbour+self)     (DVE+Pool)
    nstt      = ET_m^T @ atoms_m   (slot-major feats)       (PE)
    braw      = P_m^T @ bonds_m    (slot-major bond rows)   (PE)
    bsum      = braw summed over the 5 bond slots           (DVE)
    f0/f1/f2  = feats re-permuted slot->degree-block via
                constant permutation matmuls                (PE)
    z4/z3     = per-degree dense with W_4 / W_3 (+bias)     (PE)
    out       = relu(z)                                     (ScalarE)

  Matmul PSUM outputs may only start at partition 0/32/64, so per-molecule
  slot rows are 32-padded and grouped 3+3+2 molecules into three gather
  tiles; the dense stage uses two tiles (deg4: 96 rows, deg3: 32 rows).
  Slot capacities: 12 deg-4 + 4 deg-3 per molecule (measured maxima 12/2).
  deg<=2 atoms and any capacity overflow fall back to a tiny numpy path on
  the host (0-1 atoms in practice).  The host scatters the compact HW rows
  into the zero-initialised full output.
"""

import sys

sys.path.insert(0, "/opt/trn_rl_repo")

import numpy as np

B, A, D = 256, 128, 5
FA, FB, C = 256, 64, 256
NCORES = 8
BL = B // NCORES   # 32 molecules per core
G = 8              # molecules per group
NG = BL // G       # 4 groups per core

SJ = 32            # padded slots per molecule (16 real: 12 deg4 + 4 deg3)
SR = 16            # real slots per molecule
CAP4, CAP3 = 12, 4
Q = G * SR         # 128 dense-stage slots per group
Q3_BASE = G * CAP4  # 96: deg3 block starts here in q space
TILES3 = ((0, 3), (3, 3), (6, 2))   # (first molecule, count) per gather tile

_CACHE = {}


def _build_program():
    from contextlib import ExitStack

    import concourse.bass as bass
    import concourse.tile as tile
    from concourse import bacc, mybir

    f32 = mybir.dt.float32
    bf16 = mybir.dt.bfloat16
    AF = mybir.ActivationFunctionType
    OP = mybir.AluOpType
    AX = mybir.AxisListType

    nc = bacc.Bacc("TRN2", target_bir_lowering=False, debug=False,
                   num_devices=NCORES)

    # ---- dram tensors (host pre-laid-out per group, contiguous DMAs) ----
    atoms_d = nc.dram_tensor("atoms", [NG, A, G * FA], bf16,
                             kind="ExternalInput")
    bonds_d = nc.dram_tensor("bonds", [NG, A, G * D * FB], bf16,
                             kind="ExternalInput")
    edeg_d = nc.dram_tensor("edeg", [NG, A, G * D], bf16,
                            kind="ExternalInput")
    w0_d = nc.dram_tensor("w0", [128, 2 * C], bf16, kind="ExternalInput")
    w1_d = nc.dram_tensor("w1", [128, 2 * C], bf16, kind="ExternalInput")
    w2_d = nc.dram_tensor("w2", [FB, 2 * C], bf16, kind="ExternalInput")
    wb_d = nc.dram_tensor("wb", [1, 2 * C], bf16, kind="ExternalInput")
    ltri_d = nc.dram_tensor("ltri", [A, A], bf16, kind="ExternalInput")
    iotaj_d = nc.dram_tensor("iotaj", [A, G * SJ], bf16,
                             kind="ExternalInput")
    ladder_d = nc.dram_tensor("ladder", [A, 2 * G], bf16,
                              kind="ExternalInput")
    ones1_d = nc.dram_tensor("ones1", [1, A], bf16, kind="ExternalInput")
    sel_d = nc.dram_tensor("sel", [D, D * A], bf16, kind="ExternalInput")
    iotac_d = nc.dram_tensor("iotac", [A, 1], f32, kind="ExternalInput")
    r0_d = nc.dram_tensor("r0", [96, Q], bf16, kind="ExternalInput")
    r1_d = nc.dram_tensor("r1", [96, Q], bf16, kind="ExternalInput")
    r2_d = nc.dram_tensor("r2", [64, Q], bf16, kind="ExternalInput")
    out_d = nc.dram_tensor("out", [NG, Q, C], bf16, kind="ExternalOutput")

    atoms_ap = atoms_d.ap()
    bonds_ap = bonds_d.ap()
    edeg_ap = edeg_d.ap()
    out_ap = out_d.ap()

    with tile.TileContext(nc) as tc, ExitStack() as ctx:
        consts = ctx.enter_context(tc.tile_pool(name="consts", bufs=1))
        pin = ctx.enter_context(tc.tile_pool(name="pin", bufs=2))
        pmid = ctx.enter_context(tc.tile_pool(name="pmid", bufs=2))
        pout = ctx.enter_context(tc.tile_pool(name="pout", bufs=2))
        ps_e = ctx.enter_context(
            tc.tile_pool(name="ps_e", bufs=1, space="PSUM"))
        ps_g = ctx.enter_context(
            tc.tile_pool(name="ps_g", bufs=1, space="PSUM"))

        # ---- constants + weights (scalar/gpsimd queues) -----------------
        ltri = consts.tile([A, A], bf16)
        nc.scalar.dma_start(out=ltri[:], in_=ltri_d.ap()[:])
        iotaj = consts.tile([A, G * SJ], bf16)
        nc.scalar.dma_start(out=iotaj[:], in_=iotaj_d.ap()[:])
        ladder = consts.tile([A, 2 * G], bf16)
        nc.scalar.dma_start(out=ladder[:], in_=ladder_d.ap()[:])
        ones1 = consts.tile([1, A], bf16)
        nc.scalar.dma_start(out=ones1[:], in_=ones1_d.ap()[:])
        sel = consts.tile([D, D * A], bf16)
        nc.scalar.dma_start(out=sel[:], in_=sel_d.ap()[:])
        iotac = consts.tile([A, 1], f32)
        nc.scalar.dma_start(out=iotac[:], in_=iotac_d.ap()[:])
        rts = []
        for name, dd, rows in (("r0", r0_d, 96), ("r1", r1_d, 96),
                               ("r2", r2_d, 64)):
            rt = consts.tile([rows, Q], bf16, tag=name)
            nc.scalar.dma_start(out=rt[:], in_=dd.ap()[:])
            rts.append(rt)
        w0 = consts.tile([128, 2 * C], bf16)
        nc.gpsimd.dma_start(out=w0[:], in_=w0_d.ap()[:])
        w1 = consts.tile([128, 2 * C], bf16)
        nc.gpsimd.dma_start(out=w1[:], in_=w1_d.ap()[:])
        w2 = consts.tile([FB, 2 * C], bf16)
        nc.gpsimd.dma_start(out=w2[:], in_=w2_d.ap()[:])
        wb = consts.tile([1, 2 * C], bf16)
        nc.gpsimd.dma_start(out=wb[:], in_=wb_d.ap()[:])

        for bg in range(NG):
            # ---- inputs -------------------------------------------------
            edeg_g = pin.tile([A, G * D], bf16)
            nc.sync.dma_start(out=edeg_g[:], in_=edeg_ap[bg])
            atoms_g = pin.tile([A, G * FA], bf16)
            nc.sync.dma_start(out=atoms_g[:], in_=atoms_ap[bg])
            bonds_g = pin.tile([A, G * D * FB], bf16)
            nc.sync.dma_start(out=bonds_g[:], in_=bonds_ap[bg])

            # PSUM bank packing (8 banks, 2KB/partition each): four 1-bank
            # combined tiles + 2x2 banks for the gather tiles.
            small1 = ps_e.tile([A, 400], f32, tag="small1")
            pos2 = small1[:, 0:16]
            gatt = small1[0:5, 16:144]
            f0p = small1[:, 272:400]
            e04 = ps_e.tile([A, 4 * Q], f32, tag="e04")
            misc = ps_e.tile([A, 512], f32, tag="misc")
            e4 = misc[:, 0:Q]
            f2p = misc[0:FB, Q:Q + Q]
            z4 = misc[0:Q3_BASE, 2 * Q:2 * Q + C]
            z3t = ps_e.tile([A, 384], f32, tag="z3t")
            z3 = z3t[0:Q - Q3_BASE, 0:C]
            f1p = z3t[:, C:C + Q]

            # ---- degree + per-degree ranks (Pool: SBUF-only ops) --------
            ne = pmid.tile([A, G * D], bf16)
            nc.vector.tensor_scalar(ne[:], edeg_g[:], -1.0, None,
                                    OP.not_equal)
            deg = pmid.tile([A, G], bf16)
            with nc.allow_low_precision(reason="counts <= 5 exact in bf16"):
                nc.vector.tensor_reduce(
                    deg[:], ne.rearrange("p (g d) -> p g d", g=G),
                    axis=AX.X, op=OP.add)
            # masks2: cols [0,G) -> (deg==3), cols [G,2G) -> (deg==4)
            masks2 = pmid.tile([A, 2 * G], bf16)
            nc.vector.tensor_tensor(
                masks2.rearrange("p (d g) -> p d g", d=2),
                ladder.rearrange("p (d g) -> p d g", d=2),
                deg.unsqueeze(1).broadcast_to((A, 2, G)),
                OP.is_equal)
            nc.tensor.matmul(pos2, ltri[:], masks2[:])
            posm2 = pmid.tile([A, 2 * G], bf16)
            with nc.allow_low_precision(reason="ranks <= 128 exact in bf16"):
                nc.vector.tensor_tensor(posm2[:], pos2, masks2[:], OP.mult)

            # ---- slot one-hot P, 32-padded per molecule -----------------
            pmm = pmid.tile([A, G * SJ], bf16)
            nc.gpsimd.memset(pmm[:], 0.0)
            pv = pmm.rearrange("p (m j) -> p m j", m=G)
            iv = iotaj.rearrange("p (m j) -> p m j", m=G)
            nc.vector.tensor_tensor(
                pv[:, :, 0:CAP4], iv[:, :, 0:CAP4],
                posm2[:, G:2 * G].unsqueeze(2).broadcast_to((A, G, CAP4)),
                OP.is_equal)
            nc.vector.tensor_tensor(
                pv[:, :, CAP4:SR], iv[:, :, CAP4:SR],
                posm2[:, 0:G].unsqueeze(2).broadcast_to((A, G, CAP3)),
                OP.is_equal)

            # ---- per-slot edge ids (compact m-major), broadcast, onehot -
            for m in range(G):
                nc.tensor.matmul(gatt[:, m * SR:(m + 1) * SR],
                                 edeg_g[:, m * D:(m + 1) * D],
                                 pmm[:, m * SJ:m * SJ + SR])
            gatt_sb = pmid.tile([D, Q], bf16)
            nc.scalar.copy(gatt_sb[:], gatt)

            for j in range(4):
                nc.tensor.matmul(e04[:, j * Q:(j + 1) * Q],
                                 sel[:, j * A:(j + 1) * A], gatt_sb[:])
            nc.tensor.matmul(e4, sel[:, 4 * A:5 * A], gatt_sb[:])

            cmp = pmid.tile([A, 5 * Q], bf16)
            nc.vector.tensor_scalar(cmp[:, 0:4 * Q], e04[:], iotac[:], None,
                                    OP.is_equal)
            nc.vector.tensor_scalar(cmp[:, 4 * Q:5 * Q], e4, iotac[:],
                                    None, OP.is_equal)
            etc = pmid.tile([A, Q], bf16)
            with nc.allow_low_precision(reason="counts <= 6 exact in bf16"):
                nc.vector.tensor_reduce(
                    etc[:],
                    cmp.rearrange("p (j q) -> p q j", j=5),
                    axis=AX.X, op=OP.add)
            et = pmid.tile([A, G * SJ], bf16)
            nc.gpsimd.memset(et[:], 0.0)
            ev = et.rearrange("p (m j) -> p m j", m=G)
            with nc.allow_low_precision(reason="counts <= 6 exact in bf16"):
                nc.vector.tensor_tensor(
                    ev[:, :, 0:SR], etc.rearrange("p (m s) -> p m s", m=G),
                    pv[:, :, 0:SR], OP.add)

            # ---- slot-major gathers (3 tiles, 32-row padded molecules) --
            nstts, bsums = [], []
            for t, (m0, cnt) in enumerate(TILES3):
                rows = SJ * cnt
                nstt = ps_g.tile([96, FA], f32, tag="nstt", bufs=2)
                braw = ps_g.tile([96, D * FB], f32, tag="braw", bufs=2)
                for k in range(cnt):
                    m = m0 + k
                    rb = SJ * k
                    nc.tensor.matmul(nstt[rb:rb + SJ, :],
                                     et[:, m * SJ:(m + 1) * SJ],
                                     atoms_g[:, m * FA:(m + 1) * FA])
                    nc.tensor.matmul(
                        braw[rb:rb + SJ, :],
                        pmm[:, m * SJ:(m + 1) * SJ],
                        bonds_g[:, m * D * FB:(m + 1) * D * FB])
                nstt_sb = pmid.tile([96, FA], bf16, tag="nstts", bufs=3)
                nc.scalar.copy(nstt_sb[0:rows, :], nstt[0:rows, :])
                bsum = pmid.tile([96, FB], bf16, tag="bsum", bufs=3)
                with nc.allow_low_precision(reason="bf16 bond sums"):
                    nc.vector.tensor_reduce(
                        bsum[0:rows, :],
                        braw[0:rows, :].rearrange("p (d f) -> p f d", d=D),
                        axis=AX.X, op=OP.add)
                nstts.append((nstt_sb, rows))
                bsums.append(bsum)

            # ---- permute slot-major -> degree-block order ---------------
            for t in range(3):
                nc.tensor.matmul(f0p, nstts[t][0][0:nstts[t][1], 0:128],
                                 rts[t][:], start=(t == 0), stop=(t == 2))
            for t in range(3):
                nc.tensor.matmul(f1p, nstts[t][0][0:nstts[t][1], 128:256],
                                 rts[t][:], start=(t == 0), stop=(t == 2))
            for t in range(3):
                nc.tensor.matmul(f2p, bsums[t][0:nstts[t][1], :],
                                 rts[t][:], start=(t == 0), stop=(t == 2))
            f0 = pmid.tile([128, Q], bf16)
            nc.scalar.copy(f0[:], f0p)
            f1 = pmid.tile([128, Q], bf16)
            nc.scalar.copy(f1[:], f1p)
            f2 = pmid.tile([FB, Q], bf16)
            nc.scalar.copy(f2[:], f2p)

            # ---- dense: deg4 block q[0,96) w cols [C,2C); deg3 [96,128) -
            for zt, q0, q1, c0 in ((z4, 0, Q3_BASE, C),
                                   (z3, Q3_BASE, Q, 0)):
                nc.tensor.matmul(zt, f0[:, q0:q1], w0[:, c0:c0 + C],
                                 start=True, stop=False)
                nc.tensor.matmul(zt, f1[:, q0:q1], w1[:, c0:c0 + C],
                                 start=False, stop=False)
                nc.tensor.matmul(zt, f2[:, q0:q1], w2[:, c0:c0 + C],
                                 start=False, stop=False)
                nc.tensor.matmul(zt, ones1[:, q0:q1], wb[:, c0:c0 + C],
                                 start=False, stop=True)

            out_sb = pout.tile([Q, C], bf16)
            nc.scalar.activation(out_sb[0:Q3_BASE, :], z4, AF.Relu)
            nc.scalar.activation(out_sb[Q3_BASE:Q, :], z3, AF.Relu)
            nc.gpsimd.dma_start(out=out_ap[bg], in_=out_sb[:])

    nc.compile()
    return nc


def _get_nc():
    if "nc" not in _CACHE:
        _CACHE["nc"] = _build_program()
    return _CACHE["nc"]


def _make_in_maps(atoms, bonds, edges, W, b):
    import ml_dtypes

    bf16 = ml_dtypes.bfloat16
    atoms = np.asarray(atoms, dtype=np.float32)
    bonds = np.asarray(bonds, dtype=np.float32)
    edges = np.asarray(edges)
    W = np.asarray(W, dtype=np.float32)
    b = np.asarray(b, dtype=np.float32)

    # group-major layouts: (core, group, A, G*feat)
    def grp(x, feat):
        return np.ascontiguousarray(
            x.reshape(NCORES, NG, G, A, feat).transpose(0, 1, 3, 2, 4)
            .reshape(NCORES, NG, A, G * feat).astype(bf16))

    atoms_h = grp(atoms, FA)
    bonds_h = grp(bonds.reshape(B, A, D * FB), D * FB)
    edeg_h = grp(edges.astype(np.float32), D)

    # weights for degrees (3, 4): cols [0,C) = deg3, [C,2C) = deg4
    waug = np.concatenate([W, b[:, None, :]], axis=1)       # (5, 321, 256)
    w34 = waug[[3, 4]]                                       # (2, 321, 256)
    w0_h = np.ascontiguousarray(
        w34[:, 0:128].transpose(1, 0, 2).reshape(128, 2 * C)).astype(bf16)
    w1_h = np.ascontiguousarray(
        w34[:, 128:256].transpose(1, 0, 2).reshape(128, 2 * C)).astype(bf16)
    w2_h = np.ascontiguousarray(
        w34[:, 256:320].transpose(1, 0, 2).reshape(FB, 2 * C)).astype(bf16)
    wb_h = np.ascontiguousarray(
        w34[:, 320:321].transpose(1, 0, 2).reshape(1, 2 * C)).astype(bf16)

    # ltri[k, m] = 1 if k <= m  (inclusive prefix sums via ltri^T @ mask)
    ltri = np.triu(np.ones((A, A), dtype=np.float32)).astype(bf16)
    iotaj_row = np.zeros(G * SJ, dtype=np.float32)
    for m in range(G):
        for j in range(SR):
            iotaj_row[m * SJ + j] = (j + 1) if j < CAP4 else (j - CAP4 + 1)
    iotaj = np.broadcast_to(iotaj_row, (A, G * SJ)).astype(bf16)
    ladder_row = np.array([3.0] * G + [4.0] * G, dtype=np.float32)
    ladder = np.broadcast_to(ladder_row, (A, 2 * G)).astype(bf16)
    ones1 = np.ones((1, A), dtype=np.float32).astype(bf16)
    sel = np.zeros((D, D * A), dtype=np.float32)
    for j in range(D):
        sel[j, j * A:(j + 1) * A] = 1.0
    sel = sel.astype(bf16)
    iotac = np.arange(A, dtype=np.float32).reshape(A, 1)

    rs = []
    for m0, cnt in TILES3:
        r = np.zeros((SJ * cnt, Q), dtype=np.float32)
        for k in range(cnt):
            m = m0 + k
            for j in range(SR):
                q = (m * CAP4 + j) if j < CAP4 else (
                    Q3_BASE + m * CAP3 + (j - CAP4))
                r[SJ * k + j, q] = 1.0
        rs.append(r.astype(bf16))

    return [
        {
            "atoms": atoms_h[c],
            "bonds": bonds_h[c],
            "edeg": edeg_h[c],
            "w0": w0_h, "w1": w1_h, "w2": w2_h, "wb": wb_h,
            "ltri": ltri, "iotaj": iotaj, "ladder": ladder,
            "ones1": ones1, "sel": sel, "iotac": iotac,
            "r0": rs[0], "r1": rs[1], "r2": rs[2],
        }
        for c in range(NCORES)
    ]


def _assemble(out_hw, atoms, bonds, edges, W, b):
    """Scatter compact HW rows into the full output; numpy fallback for
    atoms outside the static slot capacities (deg<=2 or rank overflow)."""
    atoms = np.asarray(atoms, dtype=np.float32)
    bonds = np.asarray(bonds, dtype=np.float32)
    edges = np.asarray(edges)
    W = np.asarray(W, dtype=np.float32)
    b = np.asarray(b, dtype=np.float32)

    deg = (edges != -1).sum(-1)                         # (B, A)
    out = np.zeros((B, A, C), dtype=np.float32)
    covered = np.zeros((B, A), dtype=bool)
    gi = np.arange(B) // G                              # global group index
    ii = np.arange(B) % G                               # molecule in group

    for d, cap, base in ((4, CAP4, 0), (3, CAP3, Q3_BASE)):
        mask = deg == d
        rank = np.cumsum(mask, axis=1)
        ok = mask & (rank <= cap)
        mi, ai = np.nonzero(ok)
        q = base + ii[mi] * cap + (rank[mi, ai] - 1)
        out[mi, ai] = out_hw[gi[mi], q].astype(np.float32)
        covered |= ok

    rest = (deg < D) & ~covered
    for m, a in zip(*np.nonzero(rest)):
        e = edges[m, a]
        e = e[e >= 0]
        fa = atoms[m, a] + (atoms[m, e].sum(0) if e.size else 0.0)
        feat = np.concatenate([fa, bonds[m, a].sum(0)])
        z = feat @ W[deg[m, a]] + b[deg[m, a]]
        out[m, a] = np.maximum(z, 0.0)
    return out


def run_sharded(atoms, bonds, edges, W, b, trace=False):
    """Run on the 8 NeuronCores; returns (output, BassKernelResults)."""
    from concourse.bass_utils import run_bass_kernel_spmd

    nc = _get_nc()
    in_maps = _make_in_maps(atoms, bonds, edges, W, b)
    res = run_bass_kernel_spmd(nc, in_maps, list(range(NCORES)), trace=trace)
    out_hw = np.concatenate(
        [np.asarray(res.results[c]["out"]) for c in range(NCORES)],
        axis=0)                                          # (NCORES*NG, Q, C)
    out = _assemble(out_hw, atoms, bonds, edges, W, b)
    return out, res


def kernel(atoms, bonds, edges, W, b):
    out, _ = run_sharded(atoms, bonds, edges, W, b)
    return out
